# revision 18
# baseline (speedup 1.0000x reference)
"""Multi-head image attention on 8 TRN2 NeuronCores — bf16 pipelined rewrite.

Reference computation (per batch element b, all fp32):
    q = x @ Wq; k = x @ Wk; v = x @ Wv          # [N, D], N=D=1024
    per head h (16 heads, dh=64):
        scores_h = q_h @ k_h^T                  # [N, N], no 1/sqrt(dh) scale
        out_h    = softmax(scores_h) @ v_h
    out = concat_h(out_h)                       # [N, D]

Sharding: data-parallel over batch — B=8 batch elements, one per core.
Weights replicated, no collectives. Inputs are shipped pre-transposed
(x^T) and converted to bf16 on the host; output is fp32.

Kernel structure (per core), all matmuls bf16 (rel tolerance 2e-2 gives
plenty of headroom; measured ~1e-3):
  qT[dt] = Wq_blk^T @ xT   kT likewise          [128, 1024] per dim-tile
  v      = x @ Wv   stored [m][128, 16, 65] with a ones column per head
  scores pair (2 heads per dim-tile): head A lives in partitions 0:64,
      head B in 64:128 -> the two score matmuls occupy different PE row
      groups (tile_position auto-derived) and run CONCURRENTLY in the
      64x128 row-tiled PE mode: [128, 1024] psum per head per m-tile.
  p = exp(scores) on ScalarE, one N=1024 activation per head-m (bf16 out).
      ScalarE is the attention-phase pace-setter: (1024+352)/1.2 = 1147ns
      per activation, 128 total = 147us.
  attn@v: outT_h accumulated per n-half: [65, 512] psum over 8 m-tiles;
      row 64 (ones column) is the softmax denominator.
  finish: PE-transpose [65,128] chunks, 1/l on DVE, scaled into [128,128]
      output staging tiles, DMA per (pair, n-chunk).

Scheduling: the PE queue is in-order, so emission order = execution
order. Scores rounds (the exp producers) are emitted one per "round";
between rounds a compile-time deadline scheduler interleaves chunks of
the remaining work (qk projections for later pairs, the V projection,
attn@v passes for earlier pairs) so the PE never head-of-line blocks on
the exp pipeline and ScalarE is fed from ~20us into the kernel.

PSUM budget (8 banks of 2KB):
  sc  2 x [128,1024] f32 = 4 banks   (score tiles, exp reads these)
  av  1 x [65, 512] f32  = 1 bank    (attn@v accumulator)
  tp  2 x [128, 65] f32  = 2 banks   (finish transposes)
  pr  1 x [128, 512] f32 = 1 bank    (projection accumulator)
"""
import sys

sys.path.insert(0, "/opt/trn_rl_repo")

from contextlib import ExitStack

import numpy as np

import concourse.bacc as bacc
import concourse.tile as tile
from concourse import mybir
from concourse.bass_utils import run_bass_kernel_spmd
from concourse.masks import make_identity
from concourse import bass_isa

P = 128
N = 1024          # tokens
D = 1024          # model dim
H = 16            # heads
DH = 64           # head dim
KT = D // P       # contraction tiles
TT = N // P       # token tiles
NP = H // 2       # head pairs (= dim tiles)
F32 = mybir.dt.float32
BF = mybir.dt.bfloat16
EXP = mybir.ActivationFunctionType.Exp

P_BUFS = 39       # p-tile pool depth ([128,1024] bf16, 2KB/partition each)


def _emit(nc, tc, xT_d, wq_d, wk_d, wv_d, out_d):
    with ExitStack() as ctx:
        pp = ctx.enter_context(tc.tile_pool(name="persist", bufs=1))
        wpool = ctx.enter_context(tc.tile_pool(name="wts", bufs=1))
        ppool = ctx.enter_context(tc.tile_pool(name="pexp", bufs=P_BUFS))
        otp = ctx.enter_context(tc.tile_pool(name="otp", bufs=2))
        rp = ctx.enter_context(tc.tile_pool(name="rp", bufs=4))
        tadd = ctx.enter_context(tc.tile_pool(name="tadd", bufs=5))
        opool = ctx.enter_context(tc.tile_pool(name="oup", bufs=8))
        scp = ctx.enter_context(tc.tile_pool(name="scp", bufs=2, space="PSUM"))
        avp = ctx.enter_context(tc.tile_pool(name="avp", bufs=1, space="PSUM"))
        tpp = ctx.enter_context(tc.tile_pool(name="tpp", bufs=2, space="PSUM"))
        prp = ctx.enter_context(tc.tile_pool(name="prp", bufs=1, space="PSUM"))

        ident = pp.tile([P, P], F32, tag="ident")
        make_identity(nc, ident)
        identb = pp.tile([P, P], BF, tag="identb")
        make_identity(nc, identb)
        # preload the exp table set during the DMA-bound prologue
        warm = pp.tile([P, 8], F32, tag="warm")
        nc.scalar.activation(warm[:], ident[:, 0:8], EXP)
        zbf = pp.tile([P, P], BF, tag="zbf")
        nc.vector.memset(zbf[:], 0.0)
        onec = pp.tile([P, 1], BF, tag="onec")
        nc.vector.memset(onec[:], 1.0)

        xT = [pp.tile([P, N], BF, tag=f"xT{i}", name=f"xT{i}") for i in range(KT)]
        qT = [pp.tile([P, N], BF, tag=f"qT{i}", name=f"qT{i}") for i in range(KT)]
        kT = [pp.tile([P, N], BF, tag=f"kT{i}", name=f"kT{i}") for i in range(KT)]
        v1 = [pp.tile([P, H, DH + 1], BF, tag=f"v1{i}", name=f"v1{i}")
              for i in range(TT)]
        wq = [wpool.tile([P, N], BF, tag=f"wq{i}", name=f"wq{i}") for i in range(KT)]
        wk = [wpool.tile([P, N], BF, tag=f"wk{i}", name=f"wk{i}") for i in range(KT)]
        wv = [wpool.tile([P, N], BF, tag=f"wv{i}", name=f"wv{i}") for i in range(KT)]

        # xT/wq interleaved so the first q projection can chase the DMAs
        for kd in range(KT):
            nc.sync.dma_start(xT[kd][:], xT_d[kd * P:(kd + 1) * P, :])
            nc.sync.dma_start(wq[kd][:], wq_d[kd * P:(kd + 1) * P, :])
        for kd in range(KT):
            nc.sync.dma_start(wk[kd][:], wk_d[kd * P:(kd + 1) * P, :])
        for kd in range(KT):
            nc.sync.dma_start(wv[kd][:], wv_d[kd * P:(kd + 1) * P, :])
        for m in range(TT):
            nc.vector.memset(v1[m][:, :, DH:DH + 1], 1.0)


        # ---------- emission helpers (each call = one "chunk") ----------
        def proj_gen(wt, dst, dt, half):
            # dst[dt][:, half] = sum_kd wt[kd][:, dt]^T @ xT[kd][:, half]
            ps = prp.tile([P, 512], F32, tag="pr", name="prps")
            for kd in range(KT):
                nc.tensor.matmul(ps[:], wt[kd][:, dt * P:(dt + 1) * P],
                                 xT[kd][:, half * 512:(half + 1) * 512],
                                 start=(kd == 0), stop=(kd == KT - 1))
            nc.vector.tensor_copy(dst[dt][:, half * 512:(half + 1) * 512], ps[:])

        def v_gen(m, half):
            # v[m-rows, half-dims] = sum_kd xT[kd][:, m]^T @ wv[kd][:, half]
            ps = prp.tile([P, 512], F32, tag="pr", name="prps")
            for kd in range(KT):
                nc.tensor.matmul(ps[:], xT[kd][:, m * P:(m + 1) * P],
                                 wv[kd][:, half * 512:(half + 1) * 512],
                                 start=(kd == 0), stop=(kd == KT - 1))
            nc.vector.tensor_copy(
                v1[m][:, half * 8:(half + 1) * 8, 0:DH],
                ps[:].rearrange("p (h d) -> p h d", d=DH))

        p_tiles = {}

        def sc_round(pr, m):
            # head A stationary = [kA; 0], head B = [0; kB] (zero-padded to
            # K=128): uniform 128x128 PE mode, no tile-mode switches; one
            # F=1024 bf16 matmul per head
            psA = scp.tile([P, N], F32, tag="sc", name="psA")
            psB = scp.tile([P, N], F32, tag="sc", name="psB")
            khA = kT[pr][0:DH, m * P:(m + 1) * P]
            khB = kT[pr][DH:P, m * P:(m + 1) * P]
            for half in range(2):
                sl = slice(half * 512, (half + 1) * 512)
                nc.tensor.matmul(psA[:, sl], khA, qT[pr][0:DH, sl],
                                 start=True, stop=True)
                nc.tensor.matmul(psB[:, sl], khB, qT[pr][DH:P, sl],
                                 start=True, stop=True)
            for hh, ps in ((0, psA), (1, psB)):
                pt = ppool.tile([P, N], BF, tag="p", name="pt")
                nc.scalar.activation(pt[:], ps[:], EXP)
                p_tiles[(pr, m, hh)] = pt

        r_tiles = {}

        def l_chunk(pr, hh):
            # softmax denominator for head pr*2+hh: bf16 tree-sum of the 8
            # p-tiles on DVE, cross-partition sum on GPSIMD, diagonal gather
            # to put l[n] on partitions, reciprocal
            pts = [p_tiles[(pr, m, hh)] for m in range(TT)]
            add = mybir.AluOpType.add
            t01 = tadd.tile([P, N], BF, tag="ta", name="t01")
            nc.vector.tensor_tensor(t01[:], pts[0][:], pts[1][:], add)
            t23 = tadd.tile([P, N], BF, tag="ta", name="t23")
            nc.vector.tensor_tensor(t23[:], pts[2][:], pts[3][:], add)
            ta = tadd.tile([P, N], BF, tag="ta", name="ta")
            nc.vector.tensor_tensor(ta[:], t01[:], t23[:], add)
            t45 = tadd.tile([P, N], BF, tag="ta", name="t45")
            nc.vector.tensor_tensor(t45[:], pts[4][:], pts[5][:], add)
            t67 = tadd.tile([P, N], BF, tag="ta", name="t67")
            nc.vector.tensor_tensor(t67[:], pts[6][:], pts[7][:], add)
            tb = tadd.tile([P, N], BF, tag="ta", name="tb")
            nc.vector.tensor_tensor(tb[:], t45[:], t67[:], add)
            pbar = tadd.tile([P, N], BF, tag="ta", name="pbar")
            nc.vector.tensor_tensor(pbar[:], ta[:], tb[:], add)
            # l on partitions via PE: l_chunk = pbar_chunk^T @ ones
            lps = prp.tile([P, 512], F32, tag="pr", name="lps")
            for c in range(TT):
                nc.tensor.matmul(lps[:, c:c + 1],
                                 pbar[:, c * P:(c + 1) * P], onec[:],
                                 start=True, stop=True)
            rt = rp.tile([P, 8], F32, tag="rt", name="rt")
            nc.vector.reciprocal(rt[:], lps[:, 0:TT])
            r_tiles[(pr, hh)] = rt

        def av_pair(pr, half):
            # col-tiled attn@v: head A -> psum partitions 0:64 (col group
            # 0-1), head B -> 64:127 (col group 2-3), concurrent. The two
            # outputs share one PSUM bank, and start=True clears has_written
            # bits bank-wide — so zero-fill the bank once with a zero
            # stationary, then accumulate everything with start=False.
            sl = slice(half * 512, (half + 1) * 512)
            ps = avp.tile([P, 512], F32, tag="av", name="avps")
            nc.tensor.matmul(ps[:], zbf[:], p_tiles[(pr, 0, 0)][:, sl],
                             start=True, stop=False, skip_group_check=True)
            for m in range(TT):
                last = m == TT - 1
                nc.tensor.matmul(ps[0:DH, :], v1[m][:, 2 * pr, 0:DH],
                                 p_tiles[(pr, m, 0)][:, sl],
                                 start=False, stop=False, skip_group_check=True)
                nc.tensor.matmul(ps[DH:P, :], v1[m][:, 2 * pr + 1, 0:DH],
                                 p_tiles[(pr, m, 1)][:, sl],
                                 start=False, stop=last, skip_group_check=True)
            ot = otp.tile([P, 512], BF, tag="ot", name="ot")
            nc.vector.tensor_copy(ot[:], ps[:])
            for c in range(4):
                cc = half * 4 + c
                oup = opool.tile([P, P], F32, tag="ou", name="oup")
                t = tpp.tile([P, P], BF, tag="tp", name="tps")
                nc.tensor.transpose(t[:], ot[:, c * P:(c + 1) * P], identb[:])
                nc.vector.tensor_scalar_mul(
                    oup[:, 0:DH], t[:, 0:DH], r_tiles[(pr, 0)][:, cc:cc + 1])
                nc.vector.tensor_scalar_mul(
                    oup[:, DH:P], t[:, DH:P], r_tiles[(pr, 1)][:, cc:cc + 1])
                nc.sync.dma_start(
                    out_d[cc * P:(cc + 1) * P, pr * P:(pr + 1) * P], oup[:])

        # ---------- compile-time schedule ----------
        # streams: (name, [chunk closures], earliest_round, deadline_round)
        # chunk cost estimates in PE cycles for round-budget accounting
        CH_PROJ = 4700
        CH_AV = 5300

        streams = []
        for dt in range(1, KT):
            cks = [(lambda d=dt, hf=hf: proj_gen(wq, qT, d, hf)) for hf in range(2)]
            cks += [(lambda d=dt, hf=hf: proj_gen(wk, kT, d, hf)) for hf in range(2)]
            streams.append([f"qk{dt}", cks, 0, 8 * dt, CH_PROJ])
        streams.append(
            ["V", [(lambda m=m, hf=hf: v_gen(m, hf))
                   for m in range(TT) for hf in range(2)], 0, 22, CH_PROJ])
        for pr_ in range(NP):
            cks = [(lambda p_=pr_, hh=hh: l_chunk(p_, hh)) for hh in range(2)]
            cks += [(lambda p_=pr_, hf=hf: av_pair(p_, hf)) for hf in range(2)]
            streams.append([f"av{pr_}", cks, 8 * pr_ + 8, 8 * pr_ + 22, CH_AV])

        v_stream = next(s for s in streams if s[0] == "V")

        # p-pool pressure accounting: sc round r writes allocs 2r, 2r+1;
        # av[p] chunk (hh, half) frees nothing until BOTH halves of a head
        # are emitted; conservatively: after av[p] chunk index i (0..3),
        # tiles of pair p freed = 8*i (half passes re-read the same tiles,
        # so a tile is free only after the second pass of its head).
        av_emitted = [0] * NP

        def freed_tiles():
            total = 0
            for p_ in range(NP):
                total += {0: 0, 1: 0, 2: 0, 3: 0, 4: 16}[av_emitted[p_]]
            return total

        def eligible(s, r):
            name, cks, earliest, _dl, _c = s
            if not cks:
                return False
            if r < earliest:
                return False
            if name.startswith("av") and v_stream[1]:
                return False    # av needs the V projection complete
            return True

        def pop_chunk(s):
            s[1].pop(0)()
            if s[0].startswith("av"):
                av_emitted[int(s[0][2:])] += 1

        # prologue: pair-0 projections, halves interleaved so round 0's
        # first score matmuls can start after two gens
        for hf in range(2):
            proj_gen(wq, qT, 0, hf)
            proj_gen(wk, kT, 0, hf)

        for r in range(NP * TT):
            pr_, m = divmod(r, TT)
            # deadlock guard: ensure the p-pool has room for this round's
            # two allocations before sc_round enters the PE queue
            while 2 * (r + 1) - freed_tiles() > P_BUFS:
                cands = [s for s in streams if s[0].startswith("av")
                         and eligible(s, 10 ** 9)]
                if not cands:
                    cands = [v_stream] if v_stream[1] else []
                if not cands:
                    raise RuntimeError("p-pool pressure unresolvable")
                pop_chunk(min(cands, key=lambda s: s[3]))
            sc_round(pr_, m)
            budget = 5500
            while budget > 0:
                cands = [s for s in streams if eligible(s, r)]
                if not cands:
                    break
                s = min(cands, key=lambda s: s[3])
                pop_chunk(s)
                budget -= s[4]

        # drain remaining work (late attn@v passes)
        while True:
            cands = [s for s in streams if eligible(s, 10 ** 9)]
            if not cands:
                break
            pop_chunk(min(cands, key=lambda s: s[3]))
        assert all(not s[1] for s in streams), \
            [s[0] for s in streams if s[1]]


def build(rep=1):
    nc = bacc.Bacc("TRN2", target_bir_lowering=False, debug=False, num_devices=8)
    xT_d = nc.dram_tensor("xT", [D, N], BF, kind="ExternalInput").ap()
    wq_d = nc.dram_tensor("Wq", [D, D], BF, kind="ExternalInput").ap()
    wk_d = nc.dram_tensor("Wk", [D, D], BF, kind="ExternalInput").ap()
    wv_d = nc.dram_tensor("Wv", [D, D], BF, kind="ExternalInput").ap()
    out_d = nc.dram_tensor("out", [N, D], F32, kind="ExternalOutput").ap()
    with tile.TileContext(nc) as tc:
        if rep == 1:
            _emit(nc, tc, xT_d, wq_d, wk_d, wv_d, out_d)
        else:
            with tc.For_i(0, rep, 1):
                _emit(nc, tc, xT_d, wq_d, wk_d, wv_d, out_d)
    nc.compile()
    return nc


def make_in_maps(inputs):
    import ml_dtypes
    bf16 = ml_dtypes.bfloat16
    wq = np.ascontiguousarray(inputs["Wq"]).astype(bf16)
    wk = np.ascontiguousarray(inputs["Wk"]).astype(bf16)
    wv = np.ascontiguousarray(inputs["Wv"]).astype(bf16)
    return [
        {"xT": np.ascontiguousarray(np.asarray(inputs["x"][b]).T).astype(bf16),
         "Wq": wq, "Wk": wk, "Wv": wv}
        for b in range(8)
    ]


_NC_CACHE = {}


def kernel(x, Wq, Wk, Wv):
    if "nc" not in _NC_CACHE:
        _NC_CACHE["nc"] = build()
    nc = _NC_CACHE["nc"]
    in_maps = make_in_maps({"x": x, "Wq": Wq, "Wk": Wk, "Wv": Wv})
    res = run_bass_kernel_spmd(nc, in_maps, core_ids=list(range(8)))
    return np.stack([res.results[b]["out"] for b in range(8)])


# revision 19
# speedup vs baseline: 2.7121x; 2.7121x over previous
"""Multi-head image attention on 8 TRN2 NeuronCores — bf16 pipelined rewrite.

Reference computation (per batch element b, all fp32):
    q = x @ Wq; k = x @ Wk; v = x @ Wv          # [N, D], N=D=1024
    per head h (16 heads, dh=64):
        scores_h = q_h @ k_h^T                  # [N, N], no 1/sqrt(dh) scale
        out_h    = softmax(scores_h) @ v_h
    out = concat_h(out_h)                       # [N, D]

Sharding: data-parallel over batch — B=8 batch elements, one per core.
Weights replicated, no collectives. Inputs are shipped pre-transposed
(x^T) and converted to bf16 on the host; output is fp32.

Kernel structure (per core), all matmuls bf16 (rel tolerance 2e-2 gives
plenty of headroom; measured ~1e-3):
  qT[dt] = Wq_blk^T @ xT   kT likewise          [128, 1024] per dim-tile
  v      = x @ Wv   stored [m][128, 16, 65] with a ones column per head
  scores pair (2 heads per dim-tile): head A lives in partitions 0:64,
      head B in 64:128 -> the two score matmuls occupy different PE row
      groups (tile_position auto-derived) and run CONCURRENTLY in the
      64x128 row-tiled PE mode: [128, 1024] psum per head per m-tile.
  p = exp(scores) on ScalarE, one N=1024 activation per head-m (bf16 out).
      ScalarE is the attention-phase pace-setter: (1024+352)/1.2 = 1147ns
      per activation, 128 total = 147us.
  attn@v: outT_h accumulated per n-half: [65, 512] psum over 8 m-tiles;
      row 64 (ones column) is the softmax denominator.
  finish: PE-transpose [65,128] chunks, 1/l on DVE, scaled into [128,128]
      output staging tiles, DMA per (pair, n-chunk).

Scheduling: the PE queue is in-order, so emission order = execution
order. Scores rounds (the exp producers) are emitted one per "round";
between rounds a compile-time deadline scheduler interleaves chunks of
the remaining work (qk projections for later pairs, the V projection,
attn@v passes for earlier pairs) so the PE never head-of-line blocks on
the exp pipeline and ScalarE is fed from ~20us into the kernel.

PSUM budget (8 banks of 2KB):
  sc  2 x [128,1024] f32 = 4 banks   (score tiles, exp reads these)
  av  1 x [65, 512] f32  = 1 bank    (attn@v accumulator)
  tp  2 x [128, 65] f32  = 2 banks   (finish transposes)
  pr  1 x [128, 512] f32 = 1 bank    (projection accumulator)
"""
import sys

sys.path.insert(0, "/opt/trn_rl_repo")

from contextlib import ExitStack

import numpy as np

import concourse.bacc as bacc
import concourse.tile as tile
from concourse import mybir
from concourse.bass_utils import run_bass_kernel_spmd
from concourse.masks import make_identity
from concourse import bass_isa

P = 128
N = 1024          # tokens
D = 1024          # model dim
H = 16            # heads
DH = 64           # head dim
KT = D // P       # contraction tiles
TT = N // P       # token tiles
NP = H // 2       # head pairs (= dim tiles)
F32 = mybir.dt.float32
BF = mybir.dt.bfloat16
EXP = mybir.ActivationFunctionType.Exp

P_BUFS = 38       # p-tile pool depth ([128,1024] bf16, 2KB/partition each)


def _emit(nc, tc, xT_d, wq_d, wk_d, wv_d, out_d):
    with ExitStack() as ctx:
        pp = ctx.enter_context(tc.tile_pool(name="persist", bufs=1))
        wpool = ctx.enter_context(tc.tile_pool(name="wts", bufs=1))
        ppool = ctx.enter_context(tc.tile_pool(name="pexp", bufs=P_BUFS))
        otp = ctx.enter_context(tc.tile_pool(name="otp", bufs=2))
        rp = ctx.enter_context(tc.tile_pool(name="rp", bufs=4))
        tadd = ctx.enter_context(tc.tile_pool(name="tadd", bufs=5))
        opool = ctx.enter_context(tc.tile_pool(name="oup", bufs=8))
        scp = ctx.enter_context(tc.tile_pool(name="scp", bufs=2, space="PSUM"))
        avp = ctx.enter_context(tc.tile_pool(name="avp", bufs=1, space="PSUM"))
        tpp = ctx.enter_context(tc.tile_pool(name="tpp", bufs=2, space="PSUM"))
        prp = ctx.enter_context(tc.tile_pool(name="prp", bufs=1, space="PSUM"))

        ident = pp.tile([P, P], F32, tag="ident")
        make_identity(nc, ident)
        identb = pp.tile([P, P], BF, tag="identb")
        make_identity(nc, identb)
        # preload the exp table set during the DMA-bound prologue
        warm = pp.tile([P, 8], F32, tag="warm")
        nc.scalar.activation(warm[:], ident[:, 0:8], EXP)
        zbf = pp.tile([P, P], BF, tag="zbf")
        nc.vector.memset(zbf[:], 0.0)
        onec = pp.tile([P, 1], BF, tag="onec")
        nc.vector.memset(onec[:], 1.0)

        xT = [pp.tile([P, N], BF, tag=f"xT{i}", name=f"xT{i}") for i in range(KT)]
        qT = [pp.tile([P, N], BF, tag=f"qT{i}", name=f"qT{i}") for i in range(KT)]
        kT = [pp.tile([P, N], BF, tag=f"kT{i}", name=f"kT{i}") for i in range(KT)]
        v1 = [pp.tile([P, H, DH + 1], BF, tag=f"v1{i}", name=f"v1{i}")
              for i in range(TT)]
        wq = [wpool.tile([P, N], BF, tag=f"wq{i}", name=f"wq{i}") for i in range(KT)]
        wk = [wpool.tile([P, N], BF, tag=f"wk{i}", name=f"wk{i}") for i in range(KT)]
        wv = [wpool.tile([P, N], BF, tag=f"wv{i}", name=f"wv{i}") for i in range(KT)]

        # xT/wq interleaved so the first q projection can chase the DMAs
        for kd in range(KT):
            nc.sync.dma_start(xT[kd][:], xT_d[kd * P:(kd + 1) * P, :])
            nc.sync.dma_start(wq[kd][:], wq_d[kd * P:(kd + 1) * P, :])
        for kd in range(KT):
            nc.sync.dma_start(wk[kd][:], wk_d[kd * P:(kd + 1) * P, :])
        for kd in range(KT):
            nc.sync.dma_start(wv[kd][:], wv_d[kd * P:(kd + 1) * P, :])
        for m in range(TT):
            nc.vector.memset(v1[m][:, :, DH:DH + 1], 1.0)


        # ---------- emission helpers (each call = one "chunk") ----------
        def proj_gen(wt, dst, dt, half):
            # dst[dt][:, half] = sum_kd wt[kd][:, dt]^T @ xT[kd][:, half]
            ps = prp.tile([P, 512], F32, tag="pr", name="prps")
            for kd in range(KT):
                nc.tensor.matmul(ps[:], wt[kd][:, dt * P:(dt + 1) * P],
                                 xT[kd][:, half * 512:(half + 1) * 512],
                                 start=(kd == 0), stop=(kd == KT - 1))
            nc.vector.tensor_copy(dst[dt][:, half * 512:(half + 1) * 512], ps[:])

        def v_gen(m, half):
            # v[m-rows, half-dims] = sum_kd xT[kd][:, m]^T @ wv[kd][:, half]
            ps = prp.tile([P, 512], F32, tag="pr", name="prps")
            for kd in range(KT):
                nc.tensor.matmul(ps[:], xT[kd][:, m * P:(m + 1) * P],
                                 wv[kd][:, half * 512:(half + 1) * 512],
                                 start=(kd == 0), stop=(kd == KT - 1))
            nc.vector.tensor_copy(
                v1[m][:, half * 8:(half + 1) * 8, 0:DH],
                ps[:].rearrange("p (h d) -> p h d", d=DH))

        p_tiles = {}

        def sc_round(pr, m):
            # head A stationary = [kA; 0], head B = [0; kB] (zero-padded to
            # K=128): uniform 128x128 PE mode, no tile-mode switches; one
            # F=1024 bf16 matmul per head
            psA = scp.tile([P, N], F32, tag="sc", name="psA")
            psB = scp.tile([P, N], F32, tag="sc", name="psB")
            khA = kT[pr][0:DH, m * P:(m + 1) * P]
            khB = kT[pr][DH:P, m * P:(m + 1) * P]
            for half in range(2):
                sl = slice(half * 512, (half + 1) * 512)
                nc.tensor.matmul(psA[:, sl], khA, qT[pr][0:DH, sl],
                                 start=True, stop=True)
                nc.tensor.matmul(psB[:, sl], khB, qT[pr][DH:P, sl],
                                 start=True, stop=True)
            for hh, ps in ((0, psA), (1, psB)):
                pt = ppool.tile([P, N], BF, tag="p", name="pt")
                nc.scalar.activation(pt[:], ps[:], EXP)
                p_tiles[(pr, m, hh)] = pt

        r_tiles = {}

        def l_chunk(pr, hh):
            # softmax denominator for head pr*2+hh: bf16 tree-sum of the 8
            # p-tiles on DVE, cross-partition sum on GPSIMD, diagonal gather
            # to put l[n] on partitions, reciprocal
            pts = [p_tiles[(pr, m, hh)] for m in range(TT)]
            add = mybir.AluOpType.add
            t01 = tadd.tile([P, N], BF, tag="ta", name="t01")
            nc.vector.tensor_tensor(t01[:], pts[0][:], pts[1][:], add)
            t23 = tadd.tile([P, N], BF, tag="ta", name="t23")
            nc.vector.tensor_tensor(t23[:], pts[2][:], pts[3][:], add)
            ta = tadd.tile([P, N], BF, tag="ta", name="ta")
            nc.vector.tensor_tensor(ta[:], t01[:], t23[:], add)
            t45 = tadd.tile([P, N], BF, tag="ta", name="t45")
            nc.vector.tensor_tensor(t45[:], pts[4][:], pts[5][:], add)
            t67 = tadd.tile([P, N], BF, tag="ta", name="t67")
            nc.vector.tensor_tensor(t67[:], pts[6][:], pts[7][:], add)
            tb = tadd.tile([P, N], BF, tag="ta", name="tb")
            nc.vector.tensor_tensor(tb[:], t45[:], t67[:], add)
            pbar = tadd.tile([P, N], BF, tag="ta", name="pbar")
            nc.vector.tensor_tensor(pbar[:], ta[:], tb[:], add)
            # l on partitions via PE: l_chunk = pbar_chunk^T @ ones
            lps = prp.tile([P, 512], F32, tag="pr", name="lps")
            for c in range(TT):
                nc.tensor.matmul(lps[:, c:c + 1],
                                 pbar[:, c * P:(c + 1) * P], onec[:],
                                 start=True, stop=True)
            rt = rp.tile([P, 8], F32, tag="rt", name="rt")
            nc.vector.reciprocal(rt[:], lps[:, 0:TT])
            r_tiles[(pr, hh)] = rt

        def av_pair(pr, half):
            # col-tiled attn@v: head A -> psum partitions 0:64 (col group
            # 0-1), head B -> 64:127 (col group 2-3), concurrent. The two
            # outputs share one PSUM bank, and start=True clears has_written
            # bits bank-wide — so zero-fill the bank once with a zero
            # stationary, then accumulate everything with start=False.
            sl = slice(half * 512, (half + 1) * 512)
            ps = avp.tile([P, 512], F32, tag="av", name="avps")
            nc.tensor.matmul(ps[:], zbf[:], p_tiles[(pr, 0, 0)][:, sl],
                             start=True, stop=False, skip_group_check=True)
            for m in range(TT):
                last = m == TT - 1
                nc.tensor.matmul(ps[0:DH, :], v1[m][:, 2 * pr, 0:DH],
                                 p_tiles[(pr, m, 0)][:, sl],
                                 start=False, stop=False, skip_group_check=True)
                nc.tensor.matmul(ps[DH:P, :], v1[m][:, 2 * pr + 1, 0:DH],
                                 p_tiles[(pr, m, 1)][:, sl],
                                 start=False, stop=last, skip_group_check=True)
            ot = otp.tile([P, 512], BF, tag="ot", name="ot")
            nc.vector.tensor_copy(ot[:], ps[:])
            for c in range(4):
                cc = half * 4 + c
                oup = opool.tile([P, P], F32, tag="ou", name="oup")
                t = tpp.tile([P, P], BF, tag="tp", name="tps")
                nc.tensor.transpose(t[:], ot[:, c * P:(c + 1) * P], identb[:])
                nc.vector.tensor_scalar_mul(
                    oup[:, 0:DH], t[:, 0:DH], r_tiles[(pr, 0)][:, cc:cc + 1])
                nc.vector.tensor_scalar_mul(
                    oup[:, DH:P], t[:, DH:P], r_tiles[(pr, 1)][:, cc:cc + 1])
                nc.sync.dma_start(
                    out_d[cc * P:(cc + 1) * P, pr * P:(pr + 1) * P], oup[:])

        # ---------- compile-time schedule ----------
        # streams: (name, [chunk closures], earliest_round, deadline_round)
        # chunk cost estimates in PE cycles for round-budget accounting
        CH_PROJ = 4700
        CH_AV = 5300

        streams = []
        for dt in range(1, KT):
            cks = [(lambda d=dt, hf=hf: proj_gen(wq, qT, d, hf)) for hf in range(2)]
            cks += [(lambda d=dt, hf=hf: proj_gen(wk, kT, d, hf)) for hf in range(2)]
            streams.append([f"qk{dt}", cks, 0, 8 * dt, CH_PROJ])
        streams.append(
            ["V", [(lambda m=m, hf=hf: v_gen(m, hf))
                   for m in range(TT) for hf in range(2)], 0, 22, CH_PROJ])
        for pr_ in range(NP):
            cks = [(lambda p_=pr_, hh=hh: l_chunk(p_, hh)) for hh in range(2)]
            cks += [(lambda p_=pr_, hf=hf: av_pair(p_, hf)) for hf in range(2)]
            streams.append([f"av{pr_}", cks, 8 * pr_ + 8, 8 * pr_ + 22, CH_AV])

        v_stream = next(s for s in streams if s[0] == "V")

        # p-pool pressure accounting: sc round r writes allocs 2r, 2r+1;
        # av[p] chunk (hh, half) frees nothing until BOTH halves of a head
        # are emitted; conservatively: after av[p] chunk index i (0..3),
        # tiles of pair p freed = 8*i (half passes re-read the same tiles,
        # so a tile is free only after the second pass of its head).
        av_emitted = [0] * NP

        def freed_tiles():
            total = 0
            for p_ in range(NP):
                total += {0: 0, 1: 0, 2: 0, 3: 0, 4: 16}[av_emitted[p_]]
            return total

        def eligible(s, r):
            name, cks, earliest, _dl, _c = s
            if not cks:
                return False
            if r < earliest:
                return False
            if name.startswith("av") and v_stream[1]:
                return False    # av needs the V projection complete
            return True

        def pop_chunk(s):
            s[1].pop(0)()
            if s[0].startswith("av"):
                av_emitted[int(s[0][2:])] += 1

        # prologue: pair-0 projections, halves interleaved so round 0's
        # first score matmuls can start after two gens
        for hf in range(2):
            proj_gen(wq, qT, 0, hf)
            proj_gen(wk, kT, 0, hf)

        for r in range(NP * TT):
            pr_, m = divmod(r, TT)
            # deadlock guard: ensure the p-pool has room for this round's
            # two allocations before sc_round enters the PE queue
            while 2 * (r + 1) - freed_tiles() > P_BUFS:
                cands = [s for s in streams if s[0].startswith("av")
                         and eligible(s, 10 ** 9)]
                if not cands:
                    cands = [v_stream] if v_stream[1] else []
                if not cands:
                    raise RuntimeError("p-pool pressure unresolvable")
                pop_chunk(min(cands, key=lambda s: s[3]))
            sc_round(pr_, m)
            budget = 5500
            while budget > 0:
                cands = [s for s in streams if eligible(s, r)]
                if not cands:
                    break
                s = min(cands, key=lambda s: s[3])
                pop_chunk(s)
                budget -= s[4]

        # drain remaining work (late attn@v passes)
        while True:
            cands = [s for s in streams if eligible(s, 10 ** 9)]
            if not cands:
                break
            pop_chunk(min(cands, key=lambda s: s[3]))
        assert all(not s[1] for s in streams), \
            [s[0] for s in streams if s[1]]


def build(rep=1):
    nc = bacc.Bacc("TRN2", target_bir_lowering=False, debug=False, num_devices=8)
    xT_d = nc.dram_tensor("xT", [D, N], BF, kind="ExternalInput").ap()
    wq_d = nc.dram_tensor("Wq", [D, D], BF, kind="ExternalInput").ap()
    wk_d = nc.dram_tensor("Wk", [D, D], BF, kind="ExternalInput").ap()
    wv_d = nc.dram_tensor("Wv", [D, D], BF, kind="ExternalInput").ap()
    out_d = nc.dram_tensor("out", [N, D], F32, kind="ExternalOutput").ap()
    with tile.TileContext(nc) as tc:
        if rep == 1:
            _emit(nc, tc, xT_d, wq_d, wk_d, wv_d, out_d)
        else:
            with tc.For_i(0, rep, 1):
                _emit(nc, tc, xT_d, wq_d, wk_d, wv_d, out_d)
    nc.compile()
    return nc


def make_in_maps(inputs):
    import ml_dtypes
    bf16 = ml_dtypes.bfloat16
    wq = np.ascontiguousarray(inputs["Wq"]).astype(bf16)
    wk = np.ascontiguousarray(inputs["Wk"]).astype(bf16)
    wv = np.ascontiguousarray(inputs["Wv"]).astype(bf16)
    return [
        {"xT": np.ascontiguousarray(np.asarray(inputs["x"][b]).T).astype(bf16),
         "Wq": wq, "Wk": wk, "Wv": wv}
        for b in range(8)
    ]


_NC_CACHE = {}


def kernel(x, Wq, Wk, Wv):
    if "nc" not in _NC_CACHE:
        _NC_CACHE["nc"] = build()
    nc = _NC_CACHE["nc"]
    in_maps = make_in_maps({"x": x, "Wq": Wq, "Wk": Wk, "Wv": Wv})
    res = run_bass_kernel_spmd(nc, in_maps, core_ids=list(range(8)))
    return np.stack([res.results[b]["out"] for b in range(8)])
